# revision 1
# baseline (speedup 1.0000x reference)
"""ListMLE-with-tail loss kernel for Trainium2 (Bass/Tile), 8-core data-parallel.

Full-input contract: kernel(output[1024,50000] f32, target[1024] i32,
tails[1024,50] i32, tail_len[1024] i32) -> neg_like[1024] f32.

Sharding: batch rows split 128 per core (one row per SBUF partition).
Per core the kernel streams the [128, 50000] row-slice through the scalar
engine's exp with fused per-chunk row-sum accumulation (accum_out), gathers
the 51 needed scores per row (target + reversed tails) with one indirect
DMA, and computes the tail term with a tensor_tensor_scan cumsum plus a
log-with-bias activation. Host-side preprocessing is limited to index/mask
arithmetic (gather indices, validity mask) — all touches of `output` data
happen on device.
"""

import functools

import numpy as np

import concourse.bass as bass
import concourse.bacc as bacc
import concourse.tile as tile
from concourse import mybir
from concourse.bass_utils import run_bass_kernel_spmd

B = 1024
V = 50000
T = 50
M = 8            # cores
P = B // M       # 128 rows per core = SBUF partitions
C = 6250         # free-dim chunk of the exp-sum stream
NCH = V // C     # 8 chunks
G = T + 1        # gathered scores per row: [target, reversed tails]

F32 = mybir.dt.float32
I32 = mybir.dt.int32


def _build_program() -> bass.Bass:
    nc = bacc.Bacc()
    x = nc.dram_tensor("x", [P, V], F32, kind="ExternalInput")
    gidx = nc.dram_tensor("gidx", [P, G], I32, kind="ExternalInput")
    maskr = nc.dram_tensor("maskr", [P, T], F32, kind="ExternalInput")
    loss = nc.dram_tensor("loss", [P, 1], F32, kind="ExternalOutput")

    with tile.TileContext(nc) as tc:
        with (
            tc.tile_pool(name="inp", bufs=3) as inp,
            tc.tile_pool(name="scratch", bufs=2) as scratch,
            tc.tile_pool(name="small", bufs=1) as small,
        ):
            # Small per-row tensors: gather indices, validity mask.
            gidx_t = small.tile([P, G], I32)
            nc.sync.dma_start(out=gidx_t[:], in_=gidx[:])
            maskr_t = small.tile([P, T], F32)
            nc.sync.dma_start(out=maskr_t[:], in_=maskr[:])

            # sg[p, 0] = x[p, target[p]]; sg[p, 1+t] = x[p, tails[p, T-1-t]]
            # HW indirect DMA consumes one index per partition per op (the
            # [P, G] offset-AP form silently uses only column 0), so gather
            # column-by-column: op k does sg[p, k] = x_flat[gidx[p, k]].
            sg = small.tile([P, G], F32)
            xflat = x[:].rearrange("p (v u) -> (p v) u", u=1)
            for k in range(G):
                nc.gpsimd.indirect_dma_start(
                    out=sg[:, k:k + 1],
                    out_offset=None,
                    in_=xflat,
                    in_offset=bass.IndirectOffsetOnAxis(ap=gidx_t[:, k:k + 1], axis=0),
                )
            # Funnel DMA-produced tiles through one DVE copy each so no
            # downstream instruction needs >1 cross-engine sync wait (the
            # TensorTensor encoding carries a single wait slot).
            maskr2 = small.tile([P, T], F32)
            nc.vector.tensor_copy(out=maskr2[:], in_=maskr_t[:])
            sg2 = small.tile([P, G], F32)
            nc.vector.tensor_copy(out=sg2[:], in_=sg[:])

            # Main stream: total_exp[p] = sum_v exp(x[p, v]), chunked.
            sums = small.tile([P, NCH], F32)
            for i in range(NCH):
                xt = inp.tile([P, C], F32)
                nc.sync.dma_start(out=xt[:], in_=x[:, i * C:(i + 1) * C])
                et = scratch.tile([P, C], F32, tag="exp_scratch")
                nc.scalar.activation(
                    out=et[:],
                    in_=xt[:],
                    func=mybir.ActivationFunctionType.Exp,
                    accum_out=sums[:, i:i + 1],
                )
            total = small.tile([P, 1], F32)
            nc.vector.reduce_sum(out=total[:], in_=sums[:], axis=mybir.AxisListType.X)

            # Tail term, all [P, <=51] ops.
            e_all = small.tile([P, G], F32)
            nc.scalar.activation(
                out=e_all[:], in_=sg[:], func=mybir.ActivationFunctionType.Exp
            )
            es = small.tile([P, T], F32)
            nc.vector.tensor_mul(out=es[:], in0=e_all[:, 1:G], in1=maskr2[:])
            # c[p, t] = cumsum of es along t == reference's cumsum of flipped es.
            c = small.tile([P, T], F32)
            nc.vector.tensor_tensor_scan(
                out=c[:],
                data0=es[:],
                data1=es[:],
                initial=0.0,
                op0=mybir.AluOpType.add,
                op1=mybir.AluOpType.bypass,
            )
            # others = total - exp(target_score) - sum(es); sum(es) = c[:, -1]
            others = small.tile([P, 1], F32)
            nc.vector.tensor_scalar(
                out=others[:],
                in0=total[:],
                scalar1=e_all[:, 0:1],
                scalar2=c[:, T - 1:T],
                op0=mybir.AluOpType.subtract,
                op1=mybir.AluOpType.subtract,
            )
            # lg = log(c + others)
            lg = small.tile([P, T], F32)
            nc.scalar.activation(
                out=lg[:],
                in_=c[:],
                func=mybir.ActivationFunctionType.Ln,
                bias=others[:],
            )
            wl = small.tile([P, T], F32)
            nc.vector.tensor_mul(out=wl[:], in0=lg[:], in1=maskr2[:])
            below = small.tile([P, 1], F32)
            nc.vector.reduce_sum(out=below[:], in_=wl[:], axis=mybir.AxisListType.X)
            sm = small.tile([P, T], F32)
            nc.vector.tensor_mul(out=sm[:], in0=sg2[:, 1:G], in1=maskr2[:])
            above = small.tile([P, 1], F32)
            nc.vector.reduce_sum(out=above[:], in_=sm[:], axis=mybir.AxisListType.X)

            # loss = -(target_score - log(total) + above - below)
            logtot = small.tile([P, 1], F32)
            nc.scalar.activation(
                out=logtot[:], in_=total[:], func=mybir.ActivationFunctionType.Ln
            )
            t1 = small.tile([P, 1], F32)
            nc.vector.tensor_scalar(
                out=t1[:],
                in0=logtot[:],
                scalar1=sg2[:, 0:1],
                scalar2=above[:],
                op0=mybir.AluOpType.subtract,
                op1=mybir.AluOpType.subtract,
            )
            res = small.tile([P, 1], F32)
            nc.vector.tensor_add(out=res[:], in0=t1[:], in1=below[:])
            nc.sync.dma_start(out=loss[:], in_=res[:])
    nc.finalize()  # runs the bacc passes (sync-wait splitting etc.)
    return nc


@functools.cache
def _program() -> bass.Bass:
    return _build_program()


def _prep_core_inputs(output, target, tails, tail_len, core):
    r0 = core * P
    x = np.ascontiguousarray(output[r0:r0 + P]).astype(np.float32, copy=False)
    tgt = target[r0:r0 + P].astype(np.int64)
    tls = tails[r0:r0 + P].astype(np.int64)
    tln = tail_len[r0:r0 + P].astype(np.int64)

    row = np.arange(P, dtype=np.int64)[:, None] * V
    gidx = np.empty((P, G), dtype=np.int32)
    gidx[:, 0] = (row[:, 0] + tgt).astype(np.int32)
    gidx[:, 1:] = (row + tls[:, ::-1]).astype(np.int32)
    # maskr[r, t] = 1 iff reversed-tail position t is valid: (T-1-t) < tail_len[r]
    tpos = np.arange(T - 1, -1, -1, dtype=np.int64)[None, :]
    maskr = (tpos < tln[:, None]).astype(np.float32)
    return {"x": x, "gidx": gidx, "maskr": np.ascontiguousarray(maskr)}


def kernel(output, target, tails, tail_len):
    output = np.asarray(output, dtype=np.float32)
    target = np.asarray(target)
    tails = np.asarray(tails)
    tail_len = np.asarray(tail_len)

    in_maps = [
        _prep_core_inputs(output, target, tails, tail_len, core) for core in range(M)
    ]
    out = run_bass_kernel_spmd(_program(), in_maps, core_ids=list(range(M)))
    global last_result
    last_result = out
    return np.concatenate(
        [r["loss"].reshape(P).astype(np.float32) for r in out.results]
    )


last_result = None

